# revision 37
# baseline (speedup 1.0000x reference)
"""Trainium2 Bass kernel for nn_MultiHeadAttention_5334349382389.

Sharding: 8 cores = 4 batches x 2 head-groups (4 heads each).
Core c handles batch b = c // 2, head-group g = c % 2 (heads 4g..4g+3).

Design: the softmax exp is the hard floor (~34us of ACT-engine time per
core; exp only runs on the scalar engine at 1 elem/cycle/lane). The
kernel is built to keep ACT saturated and everything else overlapped:
  - Input DMA: one packed tensor, column-region transfers in first-use
    order (wq,wk,bias -> xq -> xk -> wv,xv -> ident,wo -> edge) spread
    across the 3 DMA-capable rings (sync/SP, scalar/ACT, gpsimd).
    Transfers keep all 128 partitions (partition-sliced DMA is
    pathologically slow); biases ride as 4 extra weight columns.
  - Head pairs processed (2,3) then (0,1) so the edge matrix (only
    needed by head 0) can arrive last.
  - Per pair, a software pipeline alternates the two heads' score tiles:
    while ACT exps one head's scores the PE refills the other's.
    PSUM: 4 score banks + 4 attnV accumulator banks = all 8.
  - Scores use K=128 matmuls against zero-padded khp halves and attnV
    uses 128-wide windows into the 65-stride v-aux (ones column for the
    softmax denominator): full PE-array activity keeps the HAM clock at
    2.4 GHz (half-array matmuls get throttled to 1.2 GHz).
  - q0/k0 projections and the v projection are woven into pair B's
    early/steady slots (v borrows the score tile right before the score
    matmuls clear it); leftover attnV matmuls and pair-B normalize are
    carried into pair A's slack; pair-A normalize overlaps outproj
    ch1-partials so the PE never idles long enough to cool.
  - exp table preloaded at t=0; junk matmuls warm the PE during the
    DMA-bound start.

Host: packs/transposes/casts inputs per core, gathers
  out[b] = partial(b,0) + partial(b,1) + (bo + Wo @ bv)
(the bv term is exact because softmax rows sum to 1).
"""

import os
import sys

sys.path.insert(0, "/opt/trn_rl_repo")

import numpy as np

B, SEQ, DIN, DO = 4, 1024, 512, 512
NH_ALL, DK = 8, 64
NHC = 4            # heads per core
DH = NHC * DK      # 256 per-core projected dims
P = 128
CD = DIN // P      # 4 contraction chunks for projections
CH = DH // P       # 2 dh chunks (head pairs)
KT = SEQ // P      # 8 k-tiles
STR = 512          # q-stripe (matmul free dim)
NS = SEQ // STR    # 2 stripes
TVW = NHC * (DK + 1) + DK - 1  # 323: per-k-tile v-aux width (4x65 + 63 pad)
                               # windows are 128 wide so attnV matmuls keep the
                               # full PE array active (HAM stays at 2.4 GHz)

# one packed input tensor per core:
#   [wq | wk | bias4 | xq | xk | wv | xv | ident | wo]
NX = CD * SEQ                                    # 4096 per x tensor
C_WQ = 0
C_WK = C_WQ + CD * DH        # 1024
C_B4 = C_WK + CD * DH        # 2048
C_XQ = C_B4 + 4              # 2052
C_XK = C_XQ + NX             # 6148
C_WV = C_XK + NX             # 10244
C_XV = C_WV + CD * DH        # 11268
C_EYE = C_XV + NX            # 15364
C_WO = C_EYE + P             # 15492
NALL = C_WO + CH * DO        # 16516

COMPUTE = os.environ.get("KERNEL_COMPUTE_DT", "fp16")  # fp16 | bf16

_nc = None


def _np_dt():
    import ml_dtypes

    return {
        "fp16": np.float16,
        "bf16": ml_dtypes.bfloat16,
    }[COMPUTE]


def _build():
    global _nc
    if _nc is not None:
        return _nc
    import concourse.bacc as bacc
    import concourse.bass as bass
    import concourse.mybir as mybir
    import concourse.tile as tile

    f32 = mybir.dt.float32
    cdt = {
        "fp16": mybir.dt.float16,
        "bf16": mybir.dt.bfloat16,
    }[COMPUTE]
    Exp = mybir.ActivationFunctionType.Exp

    nc = bacc.Bacc("TRN2", target_bir_lowering=False, debug=False)

    pk_all = nc.dram_tensor("pk_all", (P, NALL), cdt, kind="ExternalInput")
    edge = nc.dram_tensor("edge", (SEQ, SEQ), cdt, kind="ExternalInput")
    outp = nc.dram_tensor("outp", (SEQ, DO), cdt, kind="ExternalOutput")

    edge_r = edge.rearrange("(t p) n -> t p n", p=P)
    out_r = outp.rearrange("(t p) n -> p t n", p=P)

    def sl(s):
        return slice(s * STR, (s + 1) * STR)


    with tile.TileContext(nc) as tc:
        with (
            tc.tile_pool(name="inp", bufs=1) as inp,
            tc.tile_pool(name="wts", bufs=1) as wts,
            tc.tile_pool(name="qkp", bufs=1) as qkp,
            tc.tile_pool(name="vhap", bufs=1) as vhap,
            tc.tile_pool(name="expp", bufs=8) as expp,
            tc.tile_pool(name="otp", bufs=1) as otp,
            tc.tile_pool(name="rrp", bufs=4) as rrp,
            tc.tile_pool(name="rbp", bufs=2) as rbp,
            tc.tile_pool(name="outsp", bufs=1) as outsp,
            tc.tile_pool(name="edgp", bufs=1) as edgp,
            # PSUM: spp = 2x [128,1024] f32 (4 banks), accp = 4x [128,512] (4 banks)
            tc.tile_pool(name="spp", bufs=2, space=bass.MemorySpace.PSUM) as spp,
            tc.tile_pool(name="accp", bufs=4, space=bass.MemorySpace.PSUM) as accp,
        ):
            # ---------------- input tiles ----------------
            tw = inp.tile([P, NALL], cdt, tag="tw")

            # junk weights from memset first: PE warmup needs no DMA at all
            jw = wts.tile([P, P], cdt, tag="jw")
            nc.gpsimd.memset(jw[:], 0.125)

            # Column-region transfers (always full 128 partitions -- partition
            # slicing hits a degenerate DMA path). First-use order, spread
            # across the 3 DMA-capable rings (sync/SP, scalar/ACT, gpsimd).
            def xfer(ring, a, b):
                ring.dma_start(out=tw[:, a:b], in_=pk_all[:, a:b])

            T3A, T3B = 1365, 2731

            def xsplit(base):
                xfer(nc.sync, base, base + T3A)
                xfer(nc.scalar, base + T3A, base + T3B)
                xfer(nc.gpsimd, base + T3B, base + NX)

            xfer(nc.sync, C_WQ, C_WK)            # wq
            xfer(nc.scalar, C_WK, C_XQ)          # wk + bias
            xsplit(C_XQ)                         # xq thirds
            xsplit(C_XK)                         # xk thirds
            xfer(nc.gpsimd, C_WV, C_XV)          # wv (early: v6/v7 weave)
            xsplit(C_XV)                         # xv thirds
            xfer(nc.scalar, C_EYE, NALL)         # ident, wo

            tb4h = tw[:, C_B4 : C_B4 + 4]
            teye = tw[:, C_EYE : C_EYE + P]
            twq = tw[:, C_WQ : C_WK].rearrange("p (c d) -> p c d", d=DH)
            twk = tw[:, C_WK : C_B4].rearrange("p (c d) -> p c d", d=DH)
            twv = tw[:, C_WV : C_XV].rearrange("p (c d) -> p c d", d=DH)
            two = tw[:, C_WO : NALL].rearrange("p (c d) -> p c d", d=DO)
            txq = tw[:, C_XQ : C_XK].rearrange("p (c n) -> p c n", n=SEQ)
            txk = tw[:, C_XK : C_WV].rearrange("p (c n) -> p c n", n=SEQ)
            txv = tw[:, C_XV : C_EYE].rearrange("p (c n) -> p c n", n=SEQ)

            # bias columns as fp32 (tensor_scalar wants f32 scalars)
            tb4 = wts.tile([P, 4], f32, tag="tb4")
            nc.vector.tensor_copy(out=tb4[:], in_=tb4h[:])

            # edge halves queued last; needed only by pair A (~35us in)
            edt = edgp.tile([P, KT, SEQ], cdt, tag="edg")
            edge_pt = edge.rearrange("(t p) n -> p t n", p=P)
            half = KT // 2
            nc.sync.dma_start(out=edt[:, 0:half, :], in_=edge_pt[:, 0:half, :])
            nc.gpsimd.dma_start(out=edt[:, half:KT, :], in_=edge_pt[:, half:KT, :])
            eds = [edt[:, kt, :] for kt in range(KT)]

            # v-aux ones columns + zero tail pad (device-side init; no DMA)
            tvha = vhap.tile([P, KT, TVW], cdt, tag="tvha")
            for h in range(NHC):
                nc.gpsimd.memset(tvha[:, :, h * (DK + 1) + DK : h * (DK + 1) + DK + 1], 1.0)
            nc.gpsimd.memset(tvha[:, :, NHC * (DK + 1) : TVW], 0.0)

            # preload the exp table set (~2.7us) while DMAs land; also HAM warmup
            wrm = expp.tile([P, SEQ], cdt, tag="expT")
            nc.scalar.activation(out=wrm[:, 0:P], in_=jw[:], func=Exp)
            jnk = accp.tile([P, STR], f32, tag="acc")

            def junk(n):
                for _ in range(n):
                    nc.tensor.matmul(
                        jnk[:, 0:P], lhsT=jw[:], rhs=jw[:], start=True, stop=True
                    )

            junk(30)



            # ---------------- projections ----------------
            # tqh per ch: head 2ch at partitions 0-63, head 2ch+1 at 64-127.
            # khp per head slot: kh at the head's 64-partition half, zeros in
            # the other half, so K=128 score matmuls keep the full PE array
            # active (HAM) while masking the other head.
            tqh = qkp.tile([P, CH, SEQ], cdt, tag="tqh")
            khp = qkp.tile([P, NHC, SEQ], cdt, tag="khp")
            nc.gpsimd.memset(khp[0:DK, 1::2, :], 0.0)
            nc.gpsimd.memset(khp[DK:P, 0::2, :], 0.0)

            def proj_q(ch):
                pt = spp.tile([P, SEQ], f32, tag="sc")
                for cd in range(CD):
                    for s in range(NS):
                        nc.tensor.matmul(
                            pt[:, sl(s)],
                            lhsT=twq[:, cd, ch * P : (ch + 1) * P],
                            rhs=txq[:, cd, sl(s)],
                            start=(cd == 0),
                            stop=(cd == CD - 1),
                        )
                nc.vector.tensor_scalar_add(
                    out=tqh[:, ch, :], in0=pt[:], scalar1=tb4[:, ch : ch + 1]
                )

            def proj_k(ch):
                pt = spp.tile([P, SEQ], f32, tag="sc")
                for cd in range(CD):
                    for s in range(NS):
                        nc.tensor.matmul(
                            pt[:, sl(s)],
                            lhsT=twk[:, cd, ch * P : (ch + 1) * P],
                            rhs=txk[:, cd, sl(s)],
                            start=(cd == 0),
                            stop=(cd == CD - 1),
                        )
                nc.vector.tensor_scalar_add(
                    out=khp[0:DK, 2 * ch, :],
                    in0=pt[0:DK, :],
                    scalar1=tb4[0:DK, 2 + ch : 3 + ch],
                )
                nc.vector.tensor_scalar_add(
                    out=khp[DK:P, 2 * ch + 1, :],
                    in0=pt[DK:P, :],
                    scalar1=tb4[DK:P, 2 + ch : 3 + ch],
                )

            # v: [s, dh] tiles written into vh_aug (65-wide per head, col 64 = 1)
            # v-proj borrows the target PSUM score tile right before the score
            # matmuls clear it (start=True), so it needs no extra PSUM bank.
            def proj_v_into(stt, st):
                for cd in range(CD):
                    nc.tensor.matmul(
                        stt[:, 0:DH],
                        lhsT=txv[:, cd, st * P : (st + 1) * P],
                        rhs=twv[:, cd, :],
                        start=(cd == 0),
                        stop=(cd == CD - 1),
                    )
                nc.vector.tensor_copy(
                    out=tvha[:, st, 0 : NHC * (DK + 1)].rearrange(
                        "p (h w) -> p h w", w=DK + 1
                    )[:, :, 0:DK],
                    in_=stt[:, 0:DH].rearrange("p (h d) -> p h d", h=NHC),
                )

            # pair B needs only q1 + k1 up front; q0/k0 are woven into pair
            # B's first slots (their psum partials use accp before any attnV
            # accumulator exists)
            proj_q(1)
            proj_k(1)

            def proj_qk0_stripe(is_k, s):
                def thunk():
                    pts = accp.tile(
                        [P, STR], f32, tag="acc", name=f"p0_{int(is_k)}_{s}"
                    )
                    wt = twk if is_k else twq
                    xt_ = txk if is_k else txq
                    for cd in range(CD):
                        nc.tensor.matmul(
                            pts[:, :],
                            lhsT=wt[:, cd, 0:P],
                            rhs=xt_[:, cd, sl(s)],
                            start=(cd == 0),
                            stop=(cd == CD - 1),
                        )
                    if is_k:
                        nc.vector.tensor_scalar_add(
                            out=khp[0:DK, 0, sl(s)],
                            in0=pts[0:DK, :],
                            scalar1=tb4[0:DK, 2:3],
                        )
                        nc.vector.tensor_scalar_add(
                            out=khp[DK:P, 1, sl(s)],
                            in0=pts[DK:P, :],
                            scalar1=tb4[DK:P, 2:3],
                        )
                    else:
                        nc.vector.tensor_scalar_add(
                            out=tqh[:, 0, sl(s)],
                            in0=pts[:, :],
                            scalar1=tb4[:, 0:1],
                        )
                return thunk

            # ---------------- attention, one head-pair at a time ----------------
            tot0 = otp.tile([P, SEQ], cdt, tag="tot0")
            tot1 = otp.tile([P, SEQ], cdt, tag="tot1")
            tots = (tot0, tot1)

            # pair-B v-projection weave slots: (slot, j) -> v k-tile index
            VWEAVE = {(2, 0): 6, (2, 1): 7, (3, 1): 0, (4, 1): 1, (5, 1): 2,
                      (6, 1): 3, (7, 1): 4, (7, 0): 5}

            def pair_body(ch, lag, carry_in=(), pre_work=()):
                # heads: he = 2ch (partitions 0-63), ho = 2ch+1 (64-127)
                # Software-pipelined so ACT never starves: PE issue order is
                #   [pre/carry], aV_he(kt-lag), [v-weave], S_he(kt), ...
                # and ACT order is exp_he(kt), exp_ho(kt) -- while one head's
                # exp runs, the other head's score tile is refilled.
                # Pair B: lag=4, v-weave (borrows the score tile before the
                # scores clear it), q0/k0 partials as pre_work.
                # Pair A: lag=2, edge injection, pair B's carried thunks.
                # attnVs left at the end return as per-head thunk lists.
                is_edge_pair = ch == 0
                weave_v = ch == 1
                pv = {}

                def scores_exp(kt, j, stt):
                    h = 2 * ch + j
                    inject = is_edge_pair and j == 0
                    for s in range(NS):
                        nc.tensor.matmul(
                            stt[:, sl(s)],
                            lhsT=khp[:, h, kt * P : (kt + 1) * P],
                            rhs=tqh[:, ch, sl(s)],
                            start=True,
                            stop=not inject,
                        )
                        if inject:
                            nc.tensor.matmul(
                                stt[:, sl(s)],
                                lhsT=teye[:],
                                rhs=eds[kt][:, sl(s)],
                                start=False,
                                stop=True,
                            )
                    te = expp.tile([P, SEQ], cdt, tag="expT")
                    nc.scalar.activation(out=te, in_=stt[:], func=Exp)
                    return te

                def attnv(kt, j, te):
                    h = 2 * ch + j
                    for s in range(NS):
                        if (j, s) not in pv:
                            pv[(j, s)] = accp.tile(
                                [P, STR], f32, tag="acc", name=f"pv{ch}_{j}_{s}"
                            )
                        nc.tensor.matmul(
                            pv[(j, s)][:, :],
                            lhsT=tvha[:, kt, h * (DK + 1) : h * (DK + 1) + P],
                            rhs=te[:, sl(s)],
                            start=(kt == 0),
                            stop=(kt == KT - 1),
                        )

                tes = {}
                carry_in = list(carry_in)
                pre_work = list(pre_work)
                for kt in range(KT):
                    for j in range(2):
                        if pre_work and kt < 2:
                            pre_work.pop(0)()
                        if kt < 3:
                            for _ in range(2):
                                if carry_in:
                                    carry_in.pop(0)()
                        stt = spp.tile([P, SEQ], f32, tag="sc")
                        if weave_v and (kt, j) in VWEAVE:
                            proj_v_into(stt, VWEAVE[(kt, j)])
                        if kt >= lag:
                            attnv(kt - lag, j, tes.pop((kt - lag, j)))
                        tes[(kt, j)] = scores_exp(kt, j, stt)
                carry = {0: [], 1: []}
                for kt in range(KT - lag, KT):
                    for j in range(2):
                        te = tes.pop((kt, j))
                        carry[j].append(lambda kt=kt, j=j, te=te: attnv(kt, j, te))

                def norm_j(j):
                    # tot[j*64:(j+1)*64, ch, :] = pv[0:64] / pv[64]
                    rr = rrp.tile([1, SEQ], f32, tag="rr", name=f"rr{ch}{j}")
                    rs = rrp.tile([1, SEQ], f32, tag="rs", name=f"rs{ch}{j}")
                    for s in range(NS):
                        nc.vector.tensor_copy(
                            out=rs[:, sl(s)], in_=pv[(j, s)][DK : DK + 1, :]
                        )
                    nc.vector.reciprocal_approx_fast(out=rr[:], in_=rs[:])
                    rb = rbp.tile([DK, SEQ], f32, tag="rb", name=f"rb{ch}{j}")
                    nc.gpsimd.partition_broadcast(rb[:], rr[:])
                    for s in range(NS):
                        nc.vector.tensor_mul(
                            tots[ch][j * DK : (j + 1) * DK, sl(s)],
                            pv[(j, s)][0:DK, :],
                            rb[:, sl(s)],
                        )

                norms = [lambda j=j: norm_j(j) for j in range(2)]
                return carry, norms

            pre_b = [
                proj_qk0_stripe(False, 0),
                proj_qk0_stripe(False, 1),
                proj_qk0_stripe(True, 0),
                proj_qk0_stripe(True, 1),
            ]
            carry_b, norms_b = pair_body(1, lag=4, pre_work=pre_b)
            carry_a, norms_a = pair_body(
                0, lag=2, carry_in=carry_b[0] + carry_b[1] + norms_b
            )

            # ---------------- drain + normalize + output projection --------
            # outproj ch1-partials run while the pair-A normalize chains are
            # on DVE/gpsimd, keeping the PE array active (HAM stays warm)
            oall = outsp.tile([P, KT, DO], cdt, tag="oall")

            def op_partial(po_reg, m, ch, start):
                nc.tensor.matmul(
                    po_reg,
                    lhsT=tots[ch][:, m * P : (m + 1) * P],
                    rhs=two[:, ch, :],
                    start=start,
                    stop=not start,
                )

            po_sp = {}
            for t in carry_a[0]:
                t()
            norms_a[0]()
            for g in (0, 1):
                po_sp[g] = spp.tile([P, SEQ], f32, tag="sc", name=f"po{g}")
                for mm in (0, 1):
                    op_partial(po_sp[g][:, sl(mm)], 2 * g + mm, 1, True)
                if g == 0:
                    for t in carry_a[1]:
                        t()
                    norms_a[1]()
            for m in range(4):
                reg = po_sp[m // 2][:, sl(m % 2)]
                op_partial(reg, m, 0, False)
                nc.vector.tensor_copy(out=oall[:, m, :], in_=reg)
            nc.gpsimd.dma_start(out=out_r[:, 0:4, :], in_=oall[:, 0:4, :])
            for m in range(4, KT):
                po = accp.tile([P, STR], f32, tag="acc", name=f"po{m}")
                op_partial(po[:, :], m, 1, True)
                op_partial(po[:, :], m, 0, False)
                nc.vector.tensor_copy(out=oall[:, m, :], in_=po[:, :])
                if m == 5:
                    nc.sync.dma_start(
                        out=out_r[:, 4:6, :], in_=oall[:, 4:6, :]
                    )
            nc.scalar.dma_start(out=out_r[:, 6:8, :], in_=oall[:, 6:8, :])

    nc.compile()
    _nc = nc
    return nc


def _in_maps(q, k, v, edge_matrix, Wq, bq, Wk, bk, Wv, Wo):
    dt = _np_dt()
    zeros_edge = np.zeros((SEQ, SEQ), dt)
    edge_t = np.ascontiguousarray(edge_matrix.T).astype(dt)
    ident = np.eye(P, dtype=dt)

    def re_cp(m):
        # [C*P, D] -> [P, C*D] (partition-major packing of "(c p) d -> p c d")
        cp, d = m.shape
        return np.ascontiguousarray(
            m.reshape(cp // P, P, d).transpose(1, 0, 2).reshape(P, -1)
        )

    xt = {}
    for b in range(B):
        xt[b] = (
            re_cp(np.ascontiguousarray(q[b].T).astype(dt)),
            re_cp(np.ascontiguousarray(k[b].T).astype(dt)),
            re_cp(np.ascontiguousarray(v[b].T).astype(dt)),
        )
    maps = []
    for c in range(8):
        b, g = c // 2, c % 2
        is_edge = g == 0 and b < 2
        rows = slice(g * DH, (g + 1) * DH)
        wq_c = np.ascontiguousarray(Wq[rows].T) * np.float32(1.0 / 8.0)
        bq_c = (bq[rows] * np.float32(1.0 / 8.0)).copy()
        if is_edge:
            wq_c[:, 0:DK] = 0.0
            bq_c[0:DK] = 0.0
        b4 = np.stack(
            [
                bq_c[0:P],
                bq_c[P : 2 * P],
                bk[rows][0:P],
                bk[rows][P : 2 * P],
            ],
            axis=1,
        ).astype(dt)
        pkall = np.concatenate(
            [
                re_cp(wq_c.astype(dt)),
                re_cp(np.ascontiguousarray(Wk[rows].T).astype(dt)),
                b4,
                xt[b][0],
                xt[b][1],
                re_cp(np.ascontiguousarray(Wv[rows].T).astype(dt)),
                xt[b][2],
                ident,
                re_cp(np.ascontiguousarray(Wo[:, rows].T).astype(dt)),
            ],
            axis=1,
        )
        assert pkall.shape == (P, NALL), pkall.shape
        maps.append(
            {
                "pk_all": np.ascontiguousarray(pkall),
                "edge": edge_t if is_edge else zeros_edge,
            }
        )
    return maps


def _ensure_ntff_hook():
    """Register the axon NTFF profile hook if the image's antenv lacks it."""
    import contextlib
    import ctypes
    import types

    try:
        from antenv.axon_hooks import get_axon_ntff_profile_hook  # noqa: F401
        return
    except ImportError:
        pass

    so_path = "/opt/axon/libaxon_pjrt.so"
    try:
        lib = ctypes.CDLL(so_path)
    except OSError:
        return
    if not hasattr(lib, "axon_start_nrt_profile"):
        return
    lib.axon_start_nrt_profile.argtypes = [
        ctypes.POINTER(ctypes.c_int64),
        ctypes.c_size_t,
    ]
    lib.axon_start_nrt_profile.restype = ctypes.c_int64
    lib.axon_stop_nrt_profile.argtypes = [ctypes.c_char_p]
    lib.axon_stop_nrt_profile.restype = ctypes.c_int64

    @contextlib.contextmanager
    def _hook(output_dir, device_ids):
        import jax

        jax.devices()
        if device_ids:
            ids = (ctypes.c_int64 * len(device_ids))(*device_ids)
            rc = lib.axon_start_nrt_profile(ids, len(device_ids))
        else:
            rc = lib.axon_start_nrt_profile(None, 0)
        if rc != 0:
            raise RuntimeError(f"axon_start_nrt_profile rc={rc}")
        try:
            yield
        finally:
            n = lib.axon_stop_nrt_profile(str(output_dir).encode())
            if n < 0:
                raise RuntimeError(f"axon_stop_nrt_profile rc={n}")

    _state = {"hook": _hook}
    mod = types.ModuleType("antenv.axon_hooks")
    mod.get_axon_ntff_profile_hook = lambda: _state["hook"]
    mod.set_axon_ntff_profile_hook = lambda h: _state.__setitem__("hook", h)
    import antenv

    antenv.axon_hooks = mod
    sys.modules["antenv.axon_hooks"] = mod


def kernel(q, k, v, edge_matrix, Wq, bq, Wk, bk, Wv, bv, Wo, bo, _trace=False):
    from concourse.bass_utils import run_bass_kernel_spmd

    if _trace:
        _ensure_ntff_hook()

    q, k, v = (np.asarray(t, np.float32) for t in (q, k, v))
    edge_matrix = np.asarray(edge_matrix, np.float32)
    Wq, bq, Wk, bk, Wv, bv, Wo, bo = (
        np.asarray(t, np.float32) for t in (Wq, bq, Wk, bk, Wv, bv, Wo, bo)
    )

    nc = _build()
    maps = _in_maps(q, k, v, edge_matrix, Wq, bq, Wk, bk, Wv, Wo)
    res = run_bass_kernel_spmd(nc, maps, core_ids=list(range(8)), trace=_trace)

    bo_eff = bo + Wo @ bv
    out = np.empty((B, SEQ, DO), np.float32)
    for b in range(B):
        out[b] = (
            res.results[2 * b]["outp"].astype(np.float32)
            + res.results[2 * b + 1]["outp"].astype(np.float32)
            + bo_eff
        )
    if _trace:
        return out, res
    return out
